# revision 2
# baseline (speedup 1.0000x reference)
"""CRF Viterbi decode kernel for Trainium2 (8 NeuronCores, data-parallel).

Problem: B=1024, S=512, TAGSET=50 (T=52 incl START/STOP).
Strategy:
  - Shard batch across 8 cores (128 batches/core = 128 partitions).
  - Forward pass (per core, on-device): alpha_t = max_i(alpha_{t-1,i} + trans[i,j]) + f_t[j]
    computed unmasked (mask handling folded into traceback); all 512 alpha rows
    kept in SBUF.
  - best-last candidates for every t precomputed vectorized.
  - Traceback: sequential pointer chase; the trans column gather is a one-hot
    matmul on the TensorEngine; argmax is exact first-index (including the
    reference's (alpha+trans)+f rounding order for tie fidelity).
All arithmetic matches the JAX reference bit-exactly.
"""
import sys
import types

import numpy as np

import concourse.bass as bass
import concourse.tile as tile
from concourse import mybir
from concourse.bass_utils import run_bass_kernel_spmd


def _ensure_ntff_hook():
    """The agent image's antenv lacks axon_hooks; shim it so trace=True can
    collect NTFF profiles via the ctypes hook in trn_agent_boot."""
    try:
        from antenv.axon_hooks import get_axon_ntff_profile_hook  # noqa: F401
        return
    except ImportError:
        pass
    try:
        import trn_agent_boot.trn_boot as tb
        mod = types.ModuleType('antenv.axon_hooks')
        _h = [None]
        mod.set_axon_ntff_profile_hook = lambda h: _h.__setitem__(0, h)
        mod.get_axon_ntff_profile_hook = lambda: _h[0]
        sys.modules['antenv.axon_hooks'] = mod
        mod.set_axon_ntff_profile_hook(
            tb._ntff_profile_via_ctypes('/opt/axon/libaxon_pjrt.so'))
    except Exception:
        pass


_ensure_ntff_hook()

F32 = mybir.dt.float32
I32 = mybir.dt.int32
I8 = mybir.dt.int8

# Problem constants (hardcoded per the harness contract).
B, S, TFULL = 1024, 512, 52
NT = 50                     # real tags; START/STOP can never win (margin ~1e4)
START, STOP = 50, 51
NCORES = 8
BL = B // NCORES            # 128 batches per core
BIGF = 65536.0              # iota offset for first-index argmin trick
FCH = 16                    # feats DMA chunk (timesteps per DMA)

_AluOp = mybir.AluOpType
_Axis = mybir.AxisListType

_SPLICE_N = [0]


def _split_waits(nc, max_waits=1):
    """This walrus build encodes at most one sync wait per instruction; hoist
    extra waits onto injected same-engine NoOps (engine queues are in-order,
    so semantics are preserved)."""
    for f in nc.m.functions:
        for b in f.blocks:
            insts = b.instructions
            i = 0
            while i < len(insts):
                inst = insts[i]
                si = inst.sync_info
                waits = list(si.on_wait) if si is not None and si.on_wait else []
                if len(waits) > max_waits:
                    si.on_wait = waits[-max_waits:]
                    for w in waits[:-max_waits]:
                        _SPLICE_N[0] += 1
                        nop = mybir.InstNoOp(name=f"I-wsplit{_SPLICE_N[0]}")
                        nop.engine = inst.engine
                        nop.sync_info = mybir.SyncInfo(on_wait=[w], on_update=[])
                        insts.insert(i, nop)
                        i += 1
                i += 1


def _build_program(s_len):
    """Build the per-core Bass program. Identical on all cores (SPMD)."""
    nc = bass.Bass('TRN2', target_bir_lowering=False, debug=False)

    ftime_d = nc.dram_tensor('ftime', [BL, s_len * NT], F32, kind='ExternalInput').ap()
    alpha0_d = nc.dram_tensor('alpha0', [BL, NT], F32, kind='ExternalInput').ap()
    eqt8_d = nc.dram_tensor('eqt8', [BL, s_len], I8, kind='ExternalInput').ap()
    act8_d = nc.dram_tensor('act8', [BL, s_len], I8, kind='ExternalInput').ap()
    actf_d = nc.dram_tensor('actf', [BL, s_len], F32, kind='ExternalInput').ap()
    trep_d = nc.dram_tensor('trep', [BL, NT * NT], F32, kind='ExternalInput').ap()
    tstop_d = nc.dram_tensor('tstop', [BL, NT], F32, kind='ExternalInput').ap()
    iota_d = nc.dram_tensor('iotamb', [BL, NT], F32, kind='ExternalInput').ap()
    ident_d = nc.dram_tensor('ident', [BL, BL], F32, kind='ExternalInput').ap()
    transT_d = nc.dram_tensor('transT', [NT, NT], F32, kind='ExternalInput').ap()
    dec_d = nc.dram_tensor('dec', [BL, s_len], I32, kind='ExternalOutput').ap()

    with tile.TileContext(nc) as tc:
        with tc.tile_pool(name='res', bufs=1) as res, \
             tc.tile_pool(name='fch', bufs=3) as fpool, \
             tc.tile_pool(name='cbtmp', bufs=2) as cbpool, \
             tc.tile_pool(name='tmp', bufs=2) as tmp, \
             tc.tile_pool(name='ps', bufs=2, space='PSUM') as psum:

            # ---- resident constants & state ----
            trep = res.tile([BL, NT * NT], F32, tag='trep')
            nc.gpsimd.dma_start(trep[:], trep_d[:])
            tstop = res.tile([BL, NT], F32, tag='tstop')
            nc.gpsimd.dma_start(tstop[:], tstop_d[:])
            iota = res.tile([BL, NT], F32, tag='iota')
            nc.gpsimd.dma_start(iota[:], iota_d[:])
            ident = res.tile([BL, BL], F32, tag='ident')
            nc.gpsimd.dma_start(ident[:], ident_d[:])
            transT = res.tile([NT, NT], F32, tag='transT')
            nc.gpsimd.dma_start(transT[:], transT_d[:])
            eqt8 = res.tile([BL, s_len], I8, tag='eqt8')
            nc.gpsimd.dma_start(eqt8[:], eqt8_d[:])
            act8 = res.tile([BL, s_len], I8, tag='act8')
            nc.gpsimd.dma_start(act8[:], act8_d[:])
            actf = res.tile([BL, s_len], F32, tag='actf')
            nc.gpsimd.dma_start(actf[:], actf_d[:])

            ahist = res.tile([BL, s_len * NT], F32, tag='ahist')
            nc.gpsimd.dma_start(ahist[:, 0:NT], alpha0_d[:])

            scores = res.tile([BL, NT * NT], F32, tag='scores')
            decf = res.tile([BL, s_len], F32, tag='decf')
            cball = res.tile([BL, s_len], F32, tag='cball')
            mall = res.tile([BL, s_len], F32, tag='mall')
            idx = res.tile([BL, 1], F32, tag='idx')
            idxT = res.tile([BL, 1], F32, tag='idxT')
            nc.vector.memset(idx[:], 0.0)

            # ---- forward ----
            n_ch = (s_len + FCH - 1) // FCH
            fchunks = []
            for c in range(n_ch):
                t0 = c * FCH
                t1 = min(t0 + FCH, s_len)
                ft = fpool.tile([BL, (t1 - t0) * NT], F32, tag='fch')
                nc.gpsimd.dma_start(ft[:], ftime_d[:, t0 * NT:t1 * NT])
                fchunks.append((t0, t1, ft))
                for t in range(max(t0, 1), t1):
                    aprev = ahist[:, (t - 1) * NT:t * NT]
                    nc.vector.tensor_tensor(
                        scores[:].rearrange('p (j i) -> p j i', j=NT),
                        aprev.unsqueeze(1).broadcast_to([BL, NT, NT]),
                        trep[:].rearrange('p (j i) -> p j i', j=NT),
                        op=_AluOp.add)
                    red = tmp.tile([BL, NT], F32, tag='red')
                    nc.vector.reduce_max(
                        red[:], scores[:].rearrange('p (j i) -> p j i', j=NT),
                        axis=_Axis.X)
                    nc.vector.tensor_tensor(
                        ahist[:, t * NT:(t + 1) * NT], red[:],
                        ft[:, (t - t0) * NT:(t - t0 + 1) * NT], op=_AluOp.add)

            # ---- best-last candidates, vectorized over t ----
            CBC = 64
            for t0 in range(0, s_len, CBC):
                tc_n = min(CBC, s_len - t0)
                av = ahist[:, t0 * NT:(t0 + tc_n) * NT].rearrange(
                    'p (t i) -> p t i', t=tc_n)
                cs = cbpool.tile([BL, CBC * NT], F32, tag='cs')
                csv = cs[:, 0:tc_n * NT].rearrange('p (t i) -> p t i', t=tc_n)
                nc.vector.tensor_tensor(
                    csv, av, tstop[:].unsqueeze(1).broadcast_to([BL, tc_n, NT]),
                    op=_AluOp.add)
                nc.vector.reduce_max(mall[:, t0:t0 + tc_n], csv, axis=_Axis.X)
                q = cbpool.tile([BL, CBC * NT], F32, tag='q')
                qv = q[:, 0:tc_n * NT].rearrange('p (t i) -> p t i', t=tc_n)
                nc.vector.tensor_tensor(
                    qv, csv,
                    mall[:, t0:t0 + tc_n].unsqueeze(2).broadcast_to([BL, tc_n, NT]),
                    op=_AluOp.is_equal)
                nc.vector.tensor_tensor(
                    csv, qv, iota[:].unsqueeze(1).broadcast_to([BL, tc_n, NT]),
                    op=_AluOp.mult)
                nc.vector.tensor_reduce(
                    cball[:, t0:t0 + tc_n], csv, axis=_Axis.X, op=_AluOp.min)

            # ---- traceback ----
            for c in range(n_ch - 1, -1, -1):
                t0, t1, _ = fchunks[c]
                ftb = fpool.tile([BL, (t1 - t0) * NT], F32, tag='ftb')
                nc.gpsimd.dma_start(ftb[:], ftime_d[:, t0 * NT:t1 * NT])
                for t in range(t1 - 1, t0 - 1, -1):
                    # ptr reset at t == len-1
                    nc.vector.select(idxT[:], eqt8[:, t:t + 1],
                                     cball[:, t:t + 1], idx[:])
                    # decoded[t] = (idx + BIG) * act
                    nc.vector.scalar_tensor_tensor(
                        decf[:, t:t + 1], in0=idxT[:], scalar=BIGF,
                        in1=actf[:, t:t + 1], op0=_AluOp.add, op1=_AluOp.mult)
                    if t == 0:
                        break
                    # one-hot of current pointer, transpose, gather trans column
                    oh = tmp.tile([BL, NT], F32, tag='oh')
                    nc.vector.tensor_scalar(oh[:], in0=iota[:], scalar1=idxT[:],
                                            scalar2=None, op0=_AluOp.is_equal)
                    ohT_ps = psum.tile([NT, BL], F32, tag='ohT')
                    nc.tensor.transpose(ohT_ps[:], oh[:], ident[:])
                    ohT = tmp.tile([NT, BL], F32, tag='ohTs')
                    nc.scalar.copy(ohT[:], ohT_ps[:])
                    tcol_ps = psum.tile([BL, NT], F32, tag='tcol')
                    nc.tensor.matmul(tcol_ps[:], lhsT=ohT[:], rhs=transT[:],
                                     start=True, stop=True)
                    # fval = f_t[b, ptr]
                    hf = tmp.tile([BL, NT], F32, tag='hf')
                    nc.vector.scalar_tensor_tensor(
                        hf[:], in0=iota[:], scalar=idxT[:],
                        in1=ftb[:, (t - t0) * NT:(t - t0 + 1) * NT],
                        op0=_AluOp.is_equal, op1=_AluOp.mult)
                    fval = tmp.tile([BL, 1], F32, tag='fval')
                    nc.vector.reduce_sum(fval[:], hf[:], axis=_Axis.X)
                    # s = alpha_{t-1} + trans[:, ptr]; sf = s + fval
                    s = tmp.tile([BL, NT], F32, tag='s')
                    nc.vector.tensor_tensor(
                        s[:], ahist[:, (t - 1) * NT:t * NT], tcol_ps[:],
                        op=_AluOp.add)
                    sf = tmp.tile([BL, NT], F32, tag='sf')
                    nc.vector.tensor_scalar(sf[:], in0=s[:], scalar1=fval[:],
                                            scalar2=None, op0=_AluOp.add)
                    # first-index argmax
                    m1 = tmp.tile([BL, 1], F32, tag='m1')
                    nc.vector.reduce_max(m1[:], sf[:], axis=_Axis.X)
                    q1 = tmp.tile([BL, NT], F32, tag='q1')
                    nc.vector.scalar_tensor_tensor(
                        q1[:], in0=sf[:], scalar=m1[:], in1=iota[:],
                        op0=_AluOp.is_equal, op1=_AluOp.mult)
                    idxn = tmp.tile([BL, 1], F32, tag='idxn')
                    nc.vector.tensor_reduce(idxn[:], q1[:], axis=_Axis.X,
                                            op=_AluOp.min)
                    # advance pointer where active
                    nc.vector.select(idx[:], act8[:, t:t + 1], idxn[:], idxT[:])

            deci = res.tile([BL, s_len], I32, tag='deci')
            nc.vector.tensor_copy(deci[:], decf[:])
            nc.gpsimd.dma_start(dec_d[:], deci[:])

    _split_waits(nc)
    return nc


_CACHE = {}


def _get_program(s_len):
    if s_len not in _CACHE:
        _CACHE[s_len] = _build_program(s_len)
    return _CACHE[s_len]


def kernel(feats, mask, tags, transitions, _trace=False):
    del tags  # unused by Viterbi decode
    feats = np.asarray(feats, dtype=np.float32)
    mask = np.asarray(mask)
    transitions = np.asarray(transitions, dtype=np.float32)
    b, s, tfull = feats.shape
    assert (b, tfull) == (B, TFULL)

    lengths = np.maximum(mask.astype(bool).sum(axis=1), 1).astype(np.int64)  # [B]
    lenm1 = (lengths - 1)[:, None]                                            # [B,1]
    trange = np.arange(s)[None, :]
    eqt8 = (trange == lenm1).astype(np.int8)
    act8 = (trange <= lenm1).astype(np.int8)
    actf = act8.astype(np.float32)

    fr = feats[:, :, :NT]                                    # real-tag emissions
    alpha0 = transitions[START, :NT][None, :] + fr[:, 0, :]  # [B, NT] exact
    ftime = np.ascontiguousarray(fr).reshape(B, s * NT)      # [B, s*NT] b-major

    transT = np.ascontiguousarray(transitions[:NT, :NT].T)   # transT[j,i]=trans[i,j]
    trep = np.broadcast_to(transT.reshape(1, NT * NT), (BL, NT * NT))
    trep = np.ascontiguousarray(trep)
    tstop = np.broadcast_to(transitions[:NT, STOP][None, :], (BL, NT))
    tstop = np.ascontiguousarray(tstop)
    iotamb = np.broadcast_to((np.arange(NT, dtype=np.float32) - BIGF)[None, :],
                             (BL, NT))
    iotamb = np.ascontiguousarray(iotamb)
    ident = np.eye(BL, dtype=np.float32)

    nc = _get_program(s)
    in_maps = []
    for c in range(NCORES):
        sl = slice(c * BL, (c + 1) * BL)
        in_maps.append({
            'ftime': ftime[sl], 'alpha0': np.ascontiguousarray(alpha0[sl]),
            'eqt8': np.ascontiguousarray(eqt8[sl]),
            'act8': np.ascontiguousarray(act8[sl]),
            'actf': np.ascontiguousarray(actf[sl]),
            'trep': trep, 'tstop': tstop, 'iotamb': iotamb, 'ident': ident,
            'transT': transT,
        })
    res = run_bass_kernel_spmd(nc, in_maps, list(range(NCORES)), trace=_trace)
    out = np.concatenate([res.results[c]['dec'] for c in range(NCORES)], axis=0)
    if _trace:
        kernel._last_results = res
    return out.astype(np.int32)
